# revision 19
# baseline (speedup 1.0000x reference)
"""Chamfer loss (p3 variant) on 8 Trainium2 NeuronCores.

Computes, for p, q of shape (2, 64, 1024, 4) fp32:
    d2[c,b,n,m] = ||p3[c,b,n] - q3[c,b,m]||^2   (p3 = spatial comps 1:4)
    loss = sum(min_m sqrt(max(d2,0)+1e-12)) + sum(min_n sqrt(...))

Strategy (data-parallel over batch, 8 batches per core):
  - e[n,m] = p3.q3' - 0.5|p3|^2 - 0.5|q3'|^2 = -d2/2, produced directly in
    PSUM by a matmul over embedding rows
       L = [x, y, z, -0.5*nrm, 1],  R = [x', y', z', 1, -0.5*nrm'].
  - fp32-class precision from fp16 operands WITHOUT extra matmuls: each
    embedding value v = vh + vl (hi/lo fp16 split, ~22 mantissa bits) and
    the three product terms  Lh.Rh + Lh.Rl + Ll.Rh  are contracted in a
    SINGLE K=15 matmul with stacked operands
       lhsT15 = [Lh; Lh; Ll],  rhs15 = [Rh; Rl; Rh]
    (PE matmul cost is K-independent: cost = streamed output columns, so
    K=15 costs the same 512 cycles as K=5 -- the old 3-accumulated-matmul
    hi/lo scheme did 3x the PE work for the same numerics).
  - row-min of d2 == -2 * row-max of e. A custom fused DVE op
    (MAXPAIR_REDUCE: out = max(in0,in1), accum = max-reduce) consumes the
    two 512-wide halves of a chunk-row per instruction (in0 from PSUM,
    in1 from SBUF), so every d2 element crosses the DVE at 2 elems/cycle.
    ScalarE copies one half PSUM->SBUF to enable the dual-port read.
  - both passes (p-major row-min and q-major col-min) run as independent
    matmul phases; sqrt (+2 Heron refinements) and the final sum happen on
    a [128, 256] tile of per-chunk minima.
  - software pipeline: each chunk-row allocates one 2-bank PSUM tile
    [128,1024] from a 4-deep pool (= all 8 banks), so 4 rows are in
    flight: row k+4's matmuls wait only on row k's DVE consume.

Measured on the 8-core axon TRN2 rig (HW repeat-delta timing):
  - this kernel: 357us, rel err 1.96e-06 (vs 424us for the previous
    3-matmul hi/lo + quad-row-packed variant -- same numerics).
  - engine-isolation ablations (K_EXP knobs): PE+staging only 147us;
    +ScalarE copies 211us; full 357us.  Solo issue rates: MAXPAIR
    273ns/op (2 pairs/cyc incl. PSUM operand), ScalarE copy 530ns/op.
  - a zero-dependency PE-matmul + DVE-reduce mix (K_EXP=dvemm, separate
    pools) still measures 290us vs ~125us if concurrent: PE PSUM-writes
    and DVE/ACT PSUM-reads serialize at the hardware level (plus a
    per-interleave penalty).  Total runtime is therefore approximately
    additive in PSUM traffic time: PE writes (109us) + ACT reads (136us)
    + DVE reads (69us) + startup -- which is what bounds this kernel.
    Variants that only shuffle the reduce between engines (whole-row DVE
    reduce with no ScalarE: 374us; 2x_1P-style bf16 staging; GPSIMD
    offload -- shares the DVE SBUF port) do not beat it.
"""

import os
import sys

sys.path.insert(0, "/opt/trn_rl_repo")

from contextlib import ExitStack

import numpy as np

import concourse.bass as bass
import concourse.tile as tile
from concourse import bacc, mybir

# --------------------------------------------------------------------------
# Custom DVE op: out = max(in0, in1); accum_out = max(s0, max_k out[:, k])
# --------------------------------------------------------------------------
import concourse.dve_ops as dve_ops
from concourse.dve_ops import DveOp
from concourse.dve_spec import C0, Spec, Src0, Src1, lower as dve_lower, maxx
from concourse.dve_uop import DveOpSpec


def _ref_maxpair_reduce(in0, in1, c0, c1, c2):
    b = np.maximum(in0.astype(np.float32), in1.astype(np.float32))
    P = b.shape[0]
    acc = np.maximum(
        np.broadcast_to(np.asarray(c0, np.float32), (P, 1)),
        b.reshape(P, -1).max(axis=-1, keepdims=True),
    ).astype(np.float32)
    return b, acc


def _register_maxpair():
    spec = Spec(
        body=maxx(Src0, Src1),
        accum=maxx,
        accum_init=C0,
        reference=_ref_maxpair_reduce,
    )
    shas = {}
    for ver in ("v3", "v4"):
        uops = dve_lower(spec, ver=ver)
        shas[ver] = DveOpSpec(
            name="MAXPAIR_REDUCE", opcode=0, uops=uops, rd1_en=True
        ).sha(ver)
    op = DveOp("MAXPAIR_REDUCE", spec, subdim=False, uops_sha=shas)
    if all(o.name != op.name for o in dve_ops.OPS):
        dve_ops.OPS.append(op)
        dve_ops.CUSTOM_DVE_SPECS[op.name] = spec
        dve_ops._SUB_OPCODE_FOR_NAME[op.name] = (
            max(dve_ops._SUB_OPCODE_FOR_NAME.values()) + 1
        )
        assert dve_ops._SUB_OPCODE_FOR_NAME[op.name] < 0x20
    return op


MAXPAIR_REDUCE = _register_maxpair()


def _ref_plainmax_reduce(in0, c0, c1, c2):
    b = np.maximum(in0.astype(np.float32), np.asarray(c0, np.float32))
    P = b.shape[0]
    acc = np.maximum(
        np.broadcast_to(np.asarray(c0, np.float32), (P, 1)),
        b.reshape(P, -1).max(axis=-1, keepdims=True),
    ).astype(np.float32)
    return b, acc


def _register_plainmax():
    spec = Spec(
        body=maxx(Src0, C0),
        accum=maxx,
        accum_init=C0,
        reference=lambda in0, c0, c1, c2: _ref_plainmax_reduce(in0, c0, c1, c2),
    )
    shas = {}
    for ver in ("v3", "v4"):
        uops = dve_lower(spec, ver=ver)
        shas[ver] = DveOpSpec(
            name="PLAINMAX_REDUCE", opcode=0, uops=uops, rd1_en=False
        ).sha(ver)
    op = DveOp("PLAINMAX_REDUCE", spec, subdim=False, uops_sha=shas)
    if all(o.name != op.name for o in dve_ops.OPS):
        dve_ops.OPS.append(op)
        dve_ops.CUSTOM_DVE_SPECS[op.name] = spec
        dve_ops._SUB_OPCODE_FOR_NAME[op.name] = (
            max(dve_ops._SUB_OPCODE_FOR_NAME.values()) + 1
        )
        assert dve_ops._SUB_OPCODE_FOR_NAME[op.name] < 0x20
    return op


PLAINMAX_REDUCE = _register_plainmax()

# --------------------------------------------------------------------------
# Kernel build
# --------------------------------------------------------------------------
N_CORES = 8
CH = 2  # complex channels
BPC = 8  # batches per core (64 / 8 cores)
N = 1024  # points per set
NCHUNK = N // 128  # partition chunks per batch
F32 = mybir.dt.float32
FP16 = mybir.dt.float16
NEG_SEED = -3.0e38
AX = mybir.AxisListType
ALU = mybir.AluOpType

# timing ablations: "" (real), "noact" (skip ACT copies + DVE), "nodve"
# (keep ACT copies, skip DVE), "dvesolo" (256 dependency-free MAXPAIR ops
# on fixed tiles -- pure DVE issue-rate probe), "actsolo" (same for ACT)
K_EXP = os.environ.get("K_EXP", "")
# reduce mode: "maxpair" (ACT copies stripe 1 to SBUF; DVE dual-port op).
# (A dual-PSUM-operand DVE op was tried and is rejected by the BIR verifier;
# "plainred" in K_EXP keeps the whole-row single-op variant: measured 374us
# vs maxpair's 357us.)
K_RED = os.environ.get("K_RED", "maxpair")
# pipeline-depth knobs
K_PS = int(os.environ.get("K_PS", "4"))  # 2-bank psum tiles in flight
K_IN1 = int(os.environ.get("K_IN1", "6"))  # ACT-copy staging buffers
K_SCR = int(os.environ.get("K_SCR", "4"))  # DVE dummy-out buffers
K_BATCH = int(os.environ.get("K_BATCH", "1"))  # rows per issue batch


def build_kernel(nc, repeat=1):
    p_ap = nc.dram_tensor("p", [CH, BPC, N, 4], F32, kind="ExternalInput").ap()
    q_ap = nc.dram_tensor("q", [CH, BPC, N, 4], F32, kind="ExternalInput").ap()
    out_ap = nc.dram_tensor("out", [1, 1], F32, kind="ExternalOutput").ap()
    inp = [p_ap, q_ap]

    with tile.TileContext(nc) as tc:
        with ExitStack() as ctx:
            dramp = ctx.enter_context(tc.tile_pool(name="dram", bufs=1, space="DRAM"))
            nat = ctx.enter_context(tc.tile_pool(name="nat", bufs=2))
            nrm = ctx.enter_context(tc.tile_pool(name="nrm", bufs=2))
            emb = ctx.enter_context(tc.tile_pool(name="emb", bufs=2))
            psp = ctx.enter_context(
                tc.tile_pool(name="psp", bufs=K_PS, space="PSUM")
            )
            in1p = ctx.enter_context(tc.tile_pool(name="in1p", bufs=K_IN1))
            scr = ctx.enter_context(tc.tile_pool(name="scr", bufs=K_SCR))
            fin = ctx.enter_context(tc.tile_pool(name="fin", bufs=1))

            def body(_iv=None):
                if K_EXP in ("dvesolo", "dvesolo2", "actsolo", "dvemm"):
                    # engine issue-rate probes: 256 dependency-free ops.
                    # the fixed tiles live in their own pools so the probe
                    # streams share no pool slots (no false WAR/WAW deps).
                    fixp = ctx.enter_context(
                        tc.tile_pool(name="fixp", bufs=1, space="PSUM")
                    )
                    racc = fin.tile([128, 4 * BPC * NCHUNK], F32, tag="racc")
                    ps_f = fixp.tile([128, 1024], F32, tag="psf")
                    nc.vector.memset(ps_f[:], -1.0)
                    b1_f = fin.tile([128, 512], F32, tag="b1f")
                    nc.vector.memset(b1_f[:], -2.0)
                    if K_EXP == "dvemm":
                        mme = nrm.tile([15, 1024], FP16, tag="mme")
                        nc.vector.memset(mme[:], 0.25)
                    for k in range(4 * BPC * NCHUNK):
                        if K_EXP == "dvemm":
                            # independent MM stream in other banks from the
                            # psp pool (3 x [128,1024] slots, never ps_f's)
                            mp = psp.tile([128, 1024], F32, tag="ps")
                            nc.tensor.matmul(
                                mp[:, 0:512], mme[:, 0:128], mme[:, 0:512],
                                start=True, stop=True,
                            )
                            nc.tensor.matmul(
                                mp[:, 512:1024], mme[:, 0:128], mme[:, 512:1024],
                                start=True, stop=True,
                            )
                        if K_EXP in ("dvesolo", "dvemm"):
                            sc = scr.tile([128, 512], F32, tag="sc")
                            nc.vector._custom_dve(
                                MAXPAIR_REDUCE,
                                out=sc[:],
                                in0=ps_f[:, 0:512],
                                in1=b1_f[:],
                                s0=NEG_SEED,
                                accum_out=racc[:, k : k + 1],
                            )
                        elif K_EXP == "dvesolo2":
                            sc = scr.tile([128, 512], F32, tag="sc")
                            nc.vector._custom_dve(
                                MAXPAIR_REDUCE,
                                out=sc[:],
                                in0=ps_f[:, 0:512],
                                in1=ps_f[:, 512:1024],
                                s0=NEG_SEED,
                                accum_out=racc[:, k : k + 1],
                            )
                        else:
                            b1 = in1p.tile([128, 512], F32, tag="b1")
                            nc.scalar.copy(b1[:], ps_f[:, 512:1024])
                    nc.vector.memset(racc[:], -1.0)
                    _finale(racc, 4 * BPC * NCHUNK)
                    return

                # constant rows staged via the same [128, 64] -> flat-n DMA
                # pattern as the norm rows
                ones_f32 = nrm.tile([128, 64], F32, tag="ones_f32")
                nc.vector.memset(ones_f32[:], 1.0)
                ones_nat = nrm.tile([128, 64], FP16, tag="ones_nat")
                nc.vector.tensor_copy(ones_nat[:], ones_f32[:])
                zf = nrm.tile([128, 64], F32, tag="zf")
                nc.vector.memset(zf[:], 0.0)
                zero_nat = nrm.tile([128, 64], FP16, tag="zero_nat")
                nc.vector.tensor_copy(zero_nat[:], zf[:])

                def row_view(st, row):
                    return st[row : row + 1, :].rearrange("o (p u) -> (o p) u", p=128)

                # ---- embedding staging. For each (set, ch) build fp16
                # hi/lo DRAM images of
                #   Lst = [x, y, z, -0.5*nrm, 1]   (lhsT row order)
                #   Rst = [x, y, z, 1, -0.5*nrm]   (rhs row order)
                # in flat-n point order (n = p*64+u matches the natural
                # [128, 64*4] load).
                lsts, rsts = {}, {}
                for s in range(2):
                    for c in range(CH):
                        pn = nat.tile([128, 256], F32, tag="pn")
                        nc.sync.dma_start(
                            pn[:],
                            inp[s][c].rearrange("b (x u) k -> (b x) (u k)", x=16),
                        )
                        sq = nat.tile([128, 256], F32, tag="sq")
                        nc.scalar.square(sq[:], pn[:])
                        nr = nrm.tile([128, 64], F32, tag="nr")
                        nc.vector.reduce_sum(
                            nr[:],
                            sq[:].rearrange("p (u k) -> p u k", k=4)[:, :, 1:4],
                            axis=AX.X,
                        )
                        nc.vector.tensor_scalar_mul(nr[:], nr[:], -0.5)
                        pnh = nat.tile([128, 256], FP16, tag="pnh")
                        nc.vector.tensor_copy(pnh[:], pn[:])
                        pnd = nat.tile([128, 256], F32, tag="pnd")
                        nc.vector.tensor_sub(pnd[:], pn[:], pnh[:])
                        pnl = nat.tile([128, 256], FP16, tag="pnl")
                        nc.vector.tensor_copy(pnl[:], pnd[:])
                        nrh = nrm.tile([128, 64], FP16, tag="nrh")
                        nc.vector.tensor_copy(nrh[:], nr[:])
                        nrd = nrm.tile([128, 64], F32, tag="nrd")
                        nc.vector.tensor_sub(nrd[:], nr[:], nrh[:])
                        nrl = nrm.tile([128, 64], FP16, tag="nrl")
                        nc.vector.tensor_copy(nrl[:], nrd[:])
                        for sfx, pnx, nrx, onx in (
                            ("h", pnh, nrh, ones_nat),
                            ("l", pnl, nrl, zero_nat),
                        ):
                            cr = nat.tile([128, 192], FP16, tag="cr")
                            nc.vector.tensor_copy(
                                cr[:].rearrange("p (k u) -> p k u", u=64),
                                pnx[:].rearrange("p (u k) -> p k u", k=4)[:, 1:4, :],
                            )
                            lst = dramp.tile([5, BPC * N], FP16, tag=f"lst{s}{c}{sfx}")
                            rst = dramp.tile([5, BPC * N], FP16, tag=f"rst{s}{c}{sfx}")
                            for st, nrow, orow in ((lst, 3, 4), (rst, 4, 3)):
                                nc.sync.dma_start(
                                    st[0:3, :].rearrange("k (p u) -> p k u", p=128),
                                    cr[:].rearrange("p (k u) -> p k u", u=64),
                                )
                                nc.sync.dma_start(row_view(st, nrow), nrx[:])
                                nc.sync.dma_start(row_view(st, orow), onx[:])
                            lsts[(s, c, sfx)] = lst
                            rsts[(s, c, sfx)] = rst

                # ---- accumulator of per-chunk maxima of e = -d2/2
                racc = fin.tile([128, 4 * BPC * NCHUNK], F32, tag="racc")
                if K_EXP == "noactlink":
                    b1_fix = in1p.tile([128, 512], F32, tag="b1")
                    nc.vector.memset(b1_fix[:], -2.0)

                col = 0
                for pass_ in range(2):
                    ls, rs = (0, 1) if pass_ == 0 else (1, 0)
                    for c in range(CH):
                        # K=15 stacked hi/lo operands:
                        #   L15 = [Lh; Lh; Ll],  R15 = [Rh; Rl; Rh]
                        # so one matmul contracts Lh.Rh + Lh.Rl + Ll.Rh.
                        L15 = emb.tile([15, BPC * N], FP16, tag="L15")
                        R15 = emb.tile([15, BPC * N], FP16, tag="R15")
                        nc.sync.dma_start(L15[0:5, :], lsts[(ls, c, "h")][:])
                        nc.sync.dma_start(L15[5:10, :], lsts[(ls, c, "h")][:])
                        nc.sync.dma_start(L15[10:15, :], lsts[(ls, c, "l")][:])
                        nc.sync.dma_start(R15[0:5, :], rsts[(rs, c, "h")][:])
                        nc.sync.dma_start(R15[5:10, :], rsts[(rs, c, "l")][:])
                        nc.sync.dma_start(R15[10:15, :], rsts[(rs, c, "h")][:])

                        if K_BATCH > 1 and K_EXP == "" and K_RED == "maxpair":
                            # coarse-grained issue batching: K_BATCH rows of
                            # matmuls, then their copies, then their reduces.
                            # Fewer PE<->DVE PSUM interleave switches (the
                            # hardware serializes PSUM writers vs readers).
                            for b in range(BPC):
                                for i0 in range(0, NCHUNK, K_BATCH):
                                    pss = []
                                    for i in range(i0, i0 + K_BATCH):
                                        lo = b * N + i * 128
                                        mlo = b * N
                                        ps = psp.tile([128, 1024], F32, tag="ps")
                                        nc.tensor.matmul(
                                            ps[:, 0:512],
                                            L15[:, lo : lo + 128],
                                            R15[:, mlo : mlo + 512],
                                            start=True,
                                            stop=True,
                                        )
                                        nc.tensor.matmul(
                                            ps[:, 512:1024],
                                            L15[:, lo : lo + 128],
                                            R15[:, mlo + 512 : mlo + 1024],
                                            start=True,
                                            stop=True,
                                        )
                                        pss.append(ps)
                                    b1s = []
                                    for ps in pss:
                                        b1 = in1p.tile([128, 512], F32, tag="b1")
                                        nc.scalar.copy(b1[:], ps[:, 512:1024])
                                        b1s.append(b1)
                                    for ps, b1 in zip(pss, b1s):
                                        sc = scr.tile([128, 512], F32, tag="sc")
                                        nc.vector._custom_dve(
                                            MAXPAIR_REDUCE,
                                            out=sc[:],
                                            in0=ps[:, 0:512],
                                            in1=b1[:],
                                            s0=NEG_SEED,
                                            accum_out=racc[:, col : col + 1],
                                        )
                                        col += 1
                            continue

                        for b in range(BPC):
                            for i in range(NCHUNK):
                                lo = b * N + i * 128
                                mlo = b * N
                                ps = psp.tile([128, 1024], F32, tag="ps")
                                nc.tensor.matmul(
                                    ps[:, 0:512],
                                    L15[:, lo : lo + 128],
                                    R15[:, mlo : mlo + 512],
                                    start=True,
                                    stop=True,
                                )
                                nc.tensor.matmul(
                                    ps[:, 512:1024],
                                    L15[:, lo : lo + 128],
                                    R15[:, mlo + 512 : mlo + 1024],
                                    start=True,
                                    stop=True,
                                )
                                if K_EXP == "noact":
                                    col += 1
                                    continue
                                if K_EXP == "plainred":
                                    # single-op whole-row reduce from PSUM,
                                    # no ScalarE involvement at all
                                    sc = scr.tile([128, 1024], F32, tag="sc")
                                    nc.vector._custom_dve(
                                        PLAINMAX_REDUCE,
                                        out=sc[:],
                                        in0=ps[:],
                                        s0=NEG_SEED,
                                        accum_out=racc[:, col : col + 1],
                                    )
                                    col += 1
                                    continue
                                if K_EXP == "noactlink":
                                    sc = scr.tile([128, 512], F32, tag="sc")
                                    nc.vector._custom_dve(
                                        MAXPAIR_REDUCE,
                                        out=sc[:],
                                        in0=ps[:, 0:512],
                                        in1=b1_fix[:],
                                        s0=NEG_SEED,
                                        accum_out=racc[:, col : col + 1],
                                    )
                                    col += 1
                                    continue
                                b1 = in1p.tile([128, 512], F32, tag="b1")
                                nc.scalar.copy(b1[:], ps[:, 512:1024])
                                if K_EXP == "nodve":
                                    col += 1
                                    continue
                                sc = scr.tile([128, 512], F32, tag="sc")
                                nc.vector._custom_dve(
                                    MAXPAIR_REDUCE,
                                    out=sc[:],
                                    in0=ps[:, 0:512],
                                    in1=b1[:],
                                    s0=NEG_SEED,
                                    accum_out=racc[:, col : col + 1],
                                )
                                col += 1

                # ---- finale: d2min = -2*min(racc,0); dist = sqrt(d2min+1e-12)
                # (2 Heron steps refine ScalarE's spline sqrt); sum everything.
                if K_EXP in ("nodve", "noact"):
                    nc.vector.memset(racc[:], -1.0)
                _finale(racc, col if K_EXP == "" else 4 * BPC * NCHUNK)

            def _finale(racc, ncols):
                u = fin.tile([128, ncols], F32, tag="u")
                nc.vector.tensor_scalar_min(u[:], racc[:], 0.0)
                x = fin.tile([128, ncols], F32, tag="x")
                nc.vector.tensor_scalar(x[:], u[:], -2.0, 1e-12, ALU.mult, ALU.add)
                s0t = fin.tile([128, ncols], F32, tag="s0t")
                nc.scalar.sqrt(s0t[:], x[:])
                st = s0t
                for _ in range(2):
                    r = fin.tile([128, ncols], F32, tag="r")
                    nc.vector.reciprocal(r[:], st[:])
                    t = fin.tile([128, ncols], F32, tag="t")
                    nc.vector.tensor_mul(t[:], x[:], r[:])
                    v = fin.tile([128, ncols], F32, tag="v")
                    nc.vector.tensor_add(v[:], st[:], t[:])
                    s2 = fin.tile([128, ncols], F32, tag="s2")
                    nc.vector.tensor_scalar_mul(s2[:], v[:], 0.5)
                    st = s2
                z = fin.tile([128, 1], F32, tag="z")
                nc.vector.reduce_sum(z[:], st[:], axis=AX.X)
                ones = fin.tile([128, 1], F32, tag="ones")
                nc.vector.memset(ones[:], 1.0)
                pss = psp.tile([1, 1], F32, tag="ps")
                nc.tensor.matmul(pss[:], z[:], ones[:], start=True, stop=True)
                ob = fin.tile([1, 1], F32, tag="ob")
                nc.scalar.copy(ob[:], pss[:])
                nc.sync.dma_start(out_ap[:], ob[:])

            if repeat == 1:
                body()
            else:
                with tc.For_i(0, repeat, 1) as _i:
                    body(_i)
    return nc


_CACHE = {}


def _get_compiled(repeat=1):
    if repeat not in _CACHE:
        nc = bacc.Bacc(
            "TRN2", target_bir_lowering=False, debug=False, num_devices=N_CORES
        )
        build_kernel(nc, repeat=repeat)
        nc.compile()
        _CACHE[repeat] = nc
    return _CACHE[repeat]


def kernel(p, q):
    """Full-input chamfer loss; shards batch dim over 8 NeuronCores."""
    from concourse.bass_utils import run_bass_kernel_spmd

    p = np.asarray(p, dtype=np.float32)
    q = np.asarray(q, dtype=np.float32)
    assert p.shape == (CH, N_CORES * BPC, N, 4) and q.shape == p.shape

    nc = _get_compiled(repeat=1)
    in_maps = [
        {
            "p": np.ascontiguousarray(p[:, k * BPC : (k + 1) * BPC]),
            "q": np.ascontiguousarray(q[:, k * BPC : (k + 1) * BPC]),
        }
        for k in range(N_CORES)
    ]
    res = run_bass_kernel_spmd(nc, in_maps, list(range(N_CORES)))
    total = np.float32(0.0)
    for k in range(N_CORES):
        total += np.float32(res.results[k]["out"].reshape(()))
    return np.asarray(total, dtype=np.float32).reshape(())
